# revision 7
# baseline (speedup 1.0000x reference)
"""OIM loss kernel v2 for Trainium2, 8 NeuronCores.

Sharding: data-parallel over rois. Core c handles rois [c*1024, +1024)
against the FULL bank (padded to 10752 rows = 84 tiles of 128,
replicated per core). Per core outputs, per roi: sumexp over the whole
bank and the picked logit. Host: S = out - pads, loss =
mean(mask * (ln S - picked)).

Device pipeline per core (transposed orientation: bank rows on psum
partitions, rois on the free axis):
  PE : DoubleRow fp8 matmuls  logitsT[128 bank, 1024 roi] into PSUM
       (2 x 512-wide pieces; the e4m3 bank tile is the stationary side)
  exp: route per tile:
       ACT: activation Exp psum->sbuf fp8e4 (RNE)
       DVE: tensor_scalar Schraudolph: uint8 = rint(l*8*log2e + C) whose
            bits ARE e4m3(exp(l)) to ~3%; HW convert rounds+saturates
  PE : ones DoubleRow matmuls sum slab pairs over the bank dim into 2
       per-roi-half [1,512] accumulator chains (psum partition 0 of 2
       banks; 42-pair accumulation per chain)
  picked: DVE prod[k] = inpT[k] * bselT[k] -> bf16; PE ones matmuls
       accumulate both k-chunks into chains at partition 32; drained early.
Final: DVE/ACT copy chain rows to sbuf, one strided DMA out [2, 1024].
"""

import numpy as np
import ml_dtypes

N = 8192
D = 256
L = 10532
NCORES = 8
P = 128
RPC = 1024            # rois per core
LFULL = 10752         # padded bank rows (84 tiles)
NT = 84               # bank tiles
NPAIR = NT // 2       # 42 sum pairs
IGNORE = 5554
OIM_SCALAR = 30.0
LOG2E = 1.4426950408889634

C_DVE = 55.55         # Schraudolph bias (hw convert: round+saturate)
SCL_DVE = 8.0 * LOG2E
PAD_SUM = 220.0       # pad bank rows, each contributes exactly 1.0

E4 = ml_dtypes.float8_e4m3
BF16 = ml_dtypes.bfloat16

_CACHE = {}


def routes():
    """84 tiles -> 'A' (ACT exp) or 'D' (DVE Schraudolph).
    First 3 tiles on ACT (DVE does the picked products then); ~47 A total."""
    r = {}
    acc = 0.0
    frac = 44.0 / 81.0
    for t in range(NT):
        if t < 3:
            r[t] = "A"
            continue
        acc += frac
        if acc >= 1.0:
            r[t] = "A"
            acc -= 1.0
        else:
            r[t] = "D"
    return r


ROUTES = routes()


def _build(c_dve=C_DVE, debug=False):
    import concourse.bacc as bacc
    import concourse.tile as tile
    from concourse import mybir

    f32 = mybir.dt.float32
    bf16 = mybir.dt.bfloat16
    fp8 = mybir.dt.float8e4
    u8 = mybir.dt.uint8
    AF = mybir.ActivationFunctionType
    ALU = mybir.AluOpType
    PM = mybir.MatmulPerfMode

    nc = bacc.Bacc("TRN2", target_bir_lowering=False, debug=debug,
                   enable_partition_id=False)

    d_bankT = nc.dram_tensor("bankT", [P, 2, LFULL], fp8, kind="ExternalInput").ap()
    d_inpT = nc.dram_tensor("inpT", [P, 2, RPC], fp8, kind="ExternalInput").ap()
    d_bselT = nc.dram_tensor("bselT", [P, 2, RPC], fp8, kind="ExternalInput").ap()
    d_out = nc.dram_tensor("out", [2, RPC], f32, kind="ExternalOutput").ap()

    with tile.TileContext(nc) as tc:
        with (
            tc.tile_pool(name="const", bufs=1) as const,
            tc.tile_pool(name="psum", bufs=3, space="PSUM") as psum,
            tc.tile_pool(name="psacc", bufs=1, space="PSUM") as psacc,
        ):
            # --- resident SBUF ---
            bankT_sb = const.tile([P, 2, LFULL], fp8)
            inpT_sb = const.tile([P, 2, RPC], fp8)
            bselT_sb = const.tile([P, 2, RPC], fp8)
            ES_R = 5
            es_u8 = const.tile([P, ES_R, 2, RPC], u8)
            es_f8 = es_u8.bitcast(fp8)
            ones8 = const.tile([P, 2, 16], fp8)
            ones_bf = const.tile([P, 16], bf16)
            prod = const.tile([P, 2, RPC], bf16)
            out_sb = const.tile([P, RPC], f32)
            wsrc = const.tile([P, 2, 128], fp8)

            # --- DMAs: tiny first pieces so the PE starts ~1.5us in; the
            # rest spread over queues; far bank tiles gated off the startup
            # window (each dma_start dispatch costs ~0.7us on its engine) ---
            # three parallel HWDGE queues (sync/scalar/gpsimd); ~8us of fixed
            # ring latency before any data lands, so the critical first
            # pieces are small and spread across queues
            nc.sync.dma_start(out=inpT_sb, in_=d_inpT)
            nc.sync.dma_start(out=bankT_sb[:, :, 0:2 * P],
                              in_=d_bankT[:, :, 0:2 * P])
            nc.scalar.dma_start(out=bankT_sb[:, :, 2 * P:8 * P],
                                in_=d_bankT[:, :, 2 * P:8 * P])
            nc.scalar.dma_start(out=bselT_sb, in_=d_bselT)

            # --- consts (DVE is idle at startup; keep gpsimd's FIFO for DMA) ---
            nc.vector.memset(wsrc, 0.25)
            nc.vector.memset(ones8, 1.0)
            nc.vector.memset(ones_bf, 1.0)

            nc.gpsimd.dma_start(out=bankT_sb[:, :, 8 * P:16 * P],
                                in_=d_bankT[:, :, 8 * P:16 * P])
            nc.sync.dma_start(out=bankT_sb[:, :, 16 * P:26 * P],
                              in_=d_bankT[:, :, 16 * P:26 * P])
            late_dmas = []
            for (t0, t1, anchor) in [(26, 41, 4), (41, 56, 16), (56, 70, 30),
                                     (70, 84, 44)]:
                late_dmas.append(
                    (anchor,
                     nc.gpsimd.dma_start(out=bankT_sb[:, :, t0 * P:t1 * P],
                                         in_=d_bankT[:, :, t0 * P:t1 * P])))

            # --- accumulator chains: DoubleRow matmuls require dst partition
            # 0, plain matmuls may use 32. Per roi half q, one bank holds the
            # DR sumexp chain at partition 0 and the picked chain at 32.
            acc = [psacc.tile([P, 512], f32, tag=f"acc{q}", name=f"acc{q}")
                   for q in range(2)]

            def drain(q, row, eng):
                dst = out_sb[row:row + 1, q * 512:(q + 1) * 512]
                src = acc[q][row:row + 1, :]
                if eng == "V":
                    nc.vector.tensor_copy(out=dst, in_=src)
                else:
                    nc.scalar.activation(out=dst, in_=src, func=AF.Copy)

            # --- PE warmup (p-state ramp): into the acc banks, which the
            # chains' start=True first writes later discard. Sized to bridge
            # the ~12us DMA ring latency before tile 0's data lands.
            for i in range(12):
                nc.tensor.matmul(acc[i % 2][0:128, 0:128], wsrc, wsrc,
                                 start=True, stop=True, perf_mode=PM.DoubleRow,
                                 skip_group_check=True)

            # --- main loop ---
            # Emission order = rough execution order per engine queue. Sum
            # matmuls for pair p are emitted with tile 2p+4 so the PE FIFO
            # never blocks on a slab still being exp'd; picked matmuls are
            # emitted after tile 6.
            exps = []

            # picked products first in the DVE queue: bselT arrives ~3us in,
            # both are done before the DVE's first exp needs to run
            for i in range(2):
                nc.vector.tensor_tensor(
                    out=prod[:, i, :], in0=inpT_sb[:, i, :],
                    in1=bselT_sb[:, i, :], op=ALU.mult)

            def emit_sums(pi):
                rot = pi % ES_R
                for q in range(2):
                    nc.tensor.matmul(
                        acc[q][0:1, :],
                        ones8[:, :, 0:1],
                        es_f8[:, rot, :, q * 512:(q + 1) * 512],
                        start=(pi == 0), stop=(pi == NPAIR - 1),
                        perf_mode=PM.DoubleRow,
                        skip_group_check=True)

            for t in range(NT):
                rot = (t // 2) % ES_R
                plane = t % 2
                lhsT = bankT_sb[:, :, t * P:(t + 1) * P]
                if t == 6:
                    for q in range(2):
                        for i in range(2):
                            nc.tensor.matmul(
                                acc[q][32:33, :],
                                ones_bf[:, 0:1],
                                prod[:, i, q * 512:(q + 1) * 512],
                                start=(i == 0), stop=(i == 1),
                                skip_group_check=True)
                if t == 12:
                    for q in range(2):
                        drain(q, 32, "V")
                if t >= 4 and t % 2 == 0:
                    emit_sums((t - 4) // 2)
                mm = psum.tile([P, RPC], f32, tag="mm", name=f"mm_{t}")
                for piece in range(2):
                    nc.tensor.matmul(
                        mm[:, piece * 512:(piece + 1) * 512],
                        lhsT,
                        inpT_sb[:, :, piece * 512:(piece + 1) * 512],
                        start=True, stop=True,
                        perf_mode=PM.DoubleRow)
                if ROUTES[t] == "A":
                    e = nc.scalar.activation(
                        out=es_f8[:, rot, plane, :], in_=mm, func=AF.Exp)
                else:
                    e = nc.vector.tensor_scalar(
                        out=es_u8[:, rot, plane, :],
                        in0=mm, scalar1=SCL_DVE, scalar2=c_dve,
                        op0=ALU.mult, op1=ALU.add)
                exps.append(e)
            for pi in range(NPAIR - 2, NPAIR):
                emit_sums(pi)

            for anchor, dma in late_dmas:
                tile.add_dep_helper(
                    dma.ins, exps[anchor].ins,
                    reason="keep late DMAs off the startup window")

            # --- tail: drain sum halves, one strided DMA out ---
            drain(0, 0, "V")
            drain(1, 0, "A")
            nc.sync.dma_start(out=d_out, in_=out_sb[0:64:32, :])

    nc.compile()
    return nc


def get_nc(debug=False):
    key = ("nc_v2", debug)
    if key not in _CACHE:
        _CACHE[key] = _build(debug=debug)
    return _CACHE[key]


def make_in_maps(inputs, label, ious, lut, cq, reliability):
    """Host-side shard prep: quantize, gather, transpose only."""
    inputs = np.asarray(inputs, dtype=np.float32)
    label = np.asarray(label).astype(np.int64)
    lut = np.asarray(lut, dtype=np.float32)
    cq = np.asarray(cq, dtype=np.float32)
    reliability = np.asarray(reliability, dtype=np.float32)

    bank = np.concatenate([lut, cq], axis=0)
    scaled = bank * (OIM_SCALAR * reliability)[:, None]
    sb_pad = np.zeros((LFULL, D), np.float32)
    sb_pad[:L] = scaled
    sbq = sb_pad.astype(E4)                       # [10752, 256] e4m3
    xq = inputs.astype(E4)                        # [N, 256] e4m3

    valid = label != IGNORE
    safe = np.where(valid, label, 0)
    bselq = sbq[safe]                             # [N, 256] e4m3

    bankT = np.ascontiguousarray(
        sbq.T.reshape(2, P, LFULL).transpose(1, 0, 2))        # [128,2,10752]

    in_maps = []
    for c in range(NCORES):
        r0 = c * RPC
        xb = xq[r0:r0 + RPC]                                  # [1024, 256]
        inpT = np.ascontiguousarray(
            xb.T.reshape(2, P, RPC).transpose(1, 0, 2))       # [128,2,1024]
        bselT = np.ascontiguousarray(
            bselq[r0:r0 + RPC].T.reshape(2, P, RPC).transpose(1, 0, 2))
        in_maps.append({"bankT": bankT, "inpT": inpT, "bselT": bselT})
    return in_maps


def _combine(outs, label):
    """outs: per-core [2, 1024] f32: row 0 = sumexp+pads, row 1 = picked."""
    label = np.asarray(label).astype(np.int64)
    valid = label != IGNORE
    S = np.zeros(N, np.float64)
    picked = np.zeros(N, np.float64)
    for c in range(NCORES):
        o = np.asarray(outs[c], np.float64)
        S[c * RPC:(c + 1) * RPC] = o[0]
        picked[c * RPC:(c + 1) * RPC] = o[1]
    S -= PAD_SUM
    nll = np.log(S) - picked
    nv = max(valid.sum(), 1)
    loss = (nll * valid).sum() / nv
    return np.float32(loss)


def kernel(inputs, label, ious, lut, cq, reliability):
    from concourse import bass_utils

    nc = get_nc()
    in_maps = make_in_maps(inputs, label, ious, lut, cq, reliability)
    res = bass_utils.run_bass_kernel_spmd(nc, in_maps, core_ids=list(range(NCORES)))
    return _combine([r["out"] for r in res.results], label)


# revision 10
# speedup vs baseline: 1.1847x; 1.1847x over previous
"""OIM loss kernel for Trainium2, 8 NeuronCores (fp8 transposed pipeline).

Sharding: data-parallel over rois. Core c handles rois [c*1024, +1024)
against the FULL bank (padded to 10752 rows = 84 tiles of 128,
replicated per core). Per core outputs, per roi: sumexp over the whole
bank and the picked logit. Host: S = out - pads, loss =
mean(mask * (ln S - picked)).

Device pipeline per core (transposed orientation: bank rows on psum
partitions, rois on the free axis):
  PE : DoubleRow fp8 matmuls  logitsT[128 bank, 1024 roi] into PSUM
       (2 x 512-wide pieces; the e4m3 bank tile is the stationary side)
  exp: route per tile:
       ACT: activation Exp psum->sbuf fp8e4 (RNE)
       DVE: tensor_scalar Schraudolph: uint8 = rint(l*8*log2e + C) whose
            bits ARE e4m3(exp(l)) to ~3%; HW convert rounds+saturates
  PE : ones DoubleRow matmuls sum slab pairs over the bank dim into 2
       per-roi-half [1,512] accumulator chains (psum partition 0 of 2
       banks; 42-pair accumulation per chain)
  picked: DVE prod[k] = inpT[k] * bselT[k] -> bf16; PE ones matmuls
       accumulate both k-chunks into chains at partition 32; drained early.
Final: DVE/ACT copy chain rows to sbuf, one strided DMA out [2, 1024].
"""

import numpy as np
import ml_dtypes

N = 8192
D = 256
L = 10532
NCORES = 8
P = 128
RPC = 1024            # rois per core
LFULL = 10752         # padded bank rows (84 tiles)
NT = 84               # bank tiles
NPAIR = NT // 2       # 42 sum pairs
IGNORE = 5554
OIM_SCALAR = 30.0
LOG2E = 1.4426950408889634

C_DVE = 55.55         # Schraudolph bias (hw convert: round+saturate)
SCL_DVE = 8.0 * LOG2E
PAD_SUM = 220.0       # pad bank rows, each contributes exactly 1.0

E4 = ml_dtypes.float8_e4m3
BF16 = ml_dtypes.bfloat16

_CACHE = {}


def routes():
    """84 tiles -> 'A' (ACT exp) or 'D' (DVE Schraudolph).
    First 3 tiles on ACT (DVE does the picked products then); ~47 A total."""
    r = {}
    acc = 0.0
    frac = 44.0 / 81.0
    for t in range(NT):
        if t < 3:
            r[t] = "A"
            continue
        acc += frac
        if acc >= 1.0:
            r[t] = "A"
            acc -= 1.0
        else:
            r[t] = "D"
    return r


ROUTES = routes()


def _build(c_dve=C_DVE, debug=False):
    import concourse.bacc as bacc
    import concourse.tile as tile
    from concourse import mybir

    f32 = mybir.dt.float32
    bf16 = mybir.dt.bfloat16
    fp8 = mybir.dt.float8e4
    u8 = mybir.dt.uint8
    AF = mybir.ActivationFunctionType
    ALU = mybir.AluOpType
    PM = mybir.MatmulPerfMode

    nc = bacc.Bacc("TRN2", target_bir_lowering=False, debug=debug,
                   enable_partition_id=False)

    d_bankT = nc.dram_tensor("bankT", [P, 2, LFULL], fp8, kind="ExternalInput").ap()
    d_inpT = nc.dram_tensor("inpT", [P, 2, RPC], fp8, kind="ExternalInput").ap()
    d_bselT = nc.dram_tensor("bselT", [P, 2, RPC], fp8, kind="ExternalInput").ap()
    d_out = nc.dram_tensor("out", [2, RPC], f32, kind="ExternalOutput").ap()

    with tile.TileContext(nc) as tc:
        with (
            tc.tile_pool(name="const", bufs=1) as const,
            tc.tile_pool(name="psum", bufs=3, space="PSUM") as psum,
            tc.tile_pool(name="psacc", bufs=1, space="PSUM") as psacc,
        ):
            # --- resident SBUF ---
            bankT_sb = const.tile([P, 2, LFULL], fp8)
            inpT_sb = const.tile([P, 2, RPC], fp8)
            bselT_sb = const.tile([P, 2, RPC], fp8)
            ES_R = 5
            es_u8 = const.tile([P, ES_R, 2, RPC], u8)
            es_f8 = es_u8.bitcast(fp8)
            ones8 = const.tile([P, 2, 16], fp8)
            ones_bf = const.tile([P, 16], bf16)
            prod = const.tile([P, 2, RPC], bf16)
            out_sb = const.tile([P, RPC], f32)
            wsrc = const.tile([P, 2, 128], fp8)

            # --- DMAs: tiny first pieces so the PE starts ~1.5us in; the
            # rest spread over queues; far bank tiles gated off the startup
            # window (each dma_start dispatch costs ~0.7us on its engine) ---
            # three parallel HWDGE queues (sync/scalar/gpsimd); ~8us of fixed
            # ring latency before any data lands, so tile 0's three critical
            # pieces each go first on their own queue and land concurrently
            nc.sync.dma_start(out=inpT_sb[:, :, 0:512], in_=d_inpT[:, :, 0:512])
            nc.scalar.dma_start(out=inpT_sb[:, :, 512:RPC],
                                in_=d_inpT[:, :, 512:RPC])
            nc.gpsimd.dma_start(out=bankT_sb[:, :, 0:2 * P],
                                in_=d_bankT[:, :, 0:2 * P])
            nc.sync.dma_start(out=bankT_sb[:, :, 2 * P:8 * P],
                              in_=d_bankT[:, :, 2 * P:8 * P])
            nc.scalar.dma_start(out=bselT_sb, in_=d_bselT)

            # --- consts (DVE is idle at startup; keep gpsimd's FIFO for DMA) ---
            nc.vector.memset(wsrc, 0.25)
            nc.vector.memset(ones8, 1.0)
            nc.vector.memset(ones_bf, 1.0)

            nc.gpsimd.dma_start(out=bankT_sb[:, :, 8 * P:16 * P],
                                in_=d_bankT[:, :, 8 * P:16 * P])
            nc.sync.dma_start(out=bankT_sb[:, :, 16 * P:26 * P],
                              in_=d_bankT[:, :, 16 * P:26 * P])
            late_dmas = []
            for (t0, t1, anchor) in [(26, 41, 4), (41, 56, 16), (56, 70, 30),
                                     (70, 84, 44)]:
                late_dmas.append(
                    (anchor,
                     nc.gpsimd.dma_start(out=bankT_sb[:, :, t0 * P:t1 * P],
                                         in_=d_bankT[:, :, t0 * P:t1 * P])))

            # --- accumulator chains: DoubleRow matmuls require dst partition
            # 0, plain matmuls may use 32. Per roi half q, one bank holds the
            # DR sumexp chain at partition 0 and the picked chain at 32.
            acc = [psacc.tile([P, 512], f32, tag=f"acc{q}", name=f"acc{q}")
                   for q in range(2)]

            def drain(q, row, eng):
                dst = out_sb[row:row + 1, q * 512:(q + 1) * 512]
                src = acc[q][row:row + 1, :]
                if eng == "V":
                    nc.vector.tensor_copy(out=dst, in_=src)
                else:
                    nc.scalar.activation(out=dst, in_=src, func=AF.Copy)

            # --- PE warmup (p-state ramp): into the acc banks, which the
            # chains' start=True first writes later discard. Sized to bridge
            # the ~9.5us DMA ring latency before tile 0's data lands.
            for i in range(8):
                nc.tensor.matmul(acc[i % 2][0:128, 0:128], wsrc, wsrc,
                                 start=True, stop=True, perf_mode=PM.DoubleRow,
                                 skip_group_check=True)

            # --- main loop ---
            # Emission order = rough execution order per engine queue. Sum
            # matmuls for pair p are emitted with tile 2p+4 so the PE FIFO
            # never blocks on a slab still being exp'd; picked matmuls are
            # emitted after tile 6.
            exps = []

            # picked products first in the DVE queue: bselT arrives ~3us in,
            # both are done before the DVE's first exp needs to run
            for i in range(2):
                nc.vector.tensor_tensor(
                    out=prod[:, i, :], in0=inpT_sb[:, i, :],
                    in1=bselT_sb[:, i, :], op=ALU.mult)

            def emit_sums(pi):
                rot = pi % ES_R
                for q in range(2):
                    nc.tensor.matmul(
                        acc[q][0:1, :],
                        ones8[:, :, 0:1],
                        es_f8[:, rot, :, q * 512:(q + 1) * 512],
                        start=(pi == 0), stop=(pi == NPAIR - 1),
                        perf_mode=PM.DoubleRow,
                        skip_group_check=True)

            for t in range(NT):
                rot = (t // 2) % ES_R
                plane = t % 2
                lhsT = bankT_sb[:, :, t * P:(t + 1) * P]
                if t == 6:
                    for q in range(2):
                        for i in range(2):
                            nc.tensor.matmul(
                                acc[q][32:33, :],
                                ones_bf[:, 0:1],
                                prod[:, i, q * 512:(q + 1) * 512],
                                start=(i == 0), stop=(i == 1),
                                skip_group_check=True)
                if t == 12:
                    for q in range(2):
                        drain(q, 32, "V")
                if t >= 4 and t % 2 == 0:
                    emit_sums((t - 4) // 2)
                mm = psum.tile([P, RPC], f32, tag="mm", name=f"mm_{t}")
                for piece in range(2):
                    nc.tensor.matmul(
                        mm[:, piece * 512:(piece + 1) * 512],
                        lhsT,
                        inpT_sb[:, :, piece * 512:(piece + 1) * 512],
                        start=True, stop=True,
                        perf_mode=PM.DoubleRow)
                if ROUTES[t] == "A":
                    e = nc.scalar.activation(
                        out=es_f8[:, rot, plane, :], in_=mm, func=AF.Exp)
                else:
                    e = nc.vector.tensor_scalar(
                        out=es_u8[:, rot, plane, :],
                        in0=mm, scalar1=SCL_DVE, scalar2=c_dve,
                        op0=ALU.mult, op1=ALU.add)
                exps.append(e)
            for pi in range(NPAIR - 2, NPAIR):
                emit_sums(pi)

            for anchor, dma in late_dmas:
                tile.add_dep_helper(
                    dma.ins, exps[anchor].ins,
                    reason="keep late DMAs off the startup window")

            # --- tail: drain sum halves, one strided DMA out ---
            drain(0, 0, "V")
            drain(1, 0, "A")
            nc.sync.dma_start(out=d_out, in_=out_sb[0:64:32, :])

    nc.compile()
    return nc


def get_nc(debug=False):
    key = ("nc_v2", debug)
    if key not in _CACHE:
        _CACHE[key] = _build(debug=debug)
    return _CACHE[key]


def make_in_maps(inputs, label, ious, lut, cq, reliability):
    """Host-side shard prep: quantize, gather, transpose only."""
    inputs = np.asarray(inputs, dtype=np.float32)
    label = np.asarray(label).astype(np.int64)
    lut = np.asarray(lut, dtype=np.float32)
    cq = np.asarray(cq, dtype=np.float32)
    reliability = np.asarray(reliability, dtype=np.float32)

    bank = np.concatenate([lut, cq], axis=0)
    scaled = bank * (OIM_SCALAR * reliability)[:, None]
    sb_pad = np.zeros((LFULL, D), np.float32)
    sb_pad[:L] = scaled
    sbq = sb_pad.astype(E4)                       # [10752, 256] e4m3
    xq = inputs.astype(E4)                        # [N, 256] e4m3

    valid = label != IGNORE
    safe = np.where(valid, label, 0)
    bselq = sbq[safe]                             # [N, 256] e4m3

    bankT = np.ascontiguousarray(
        sbq.T.reshape(2, P, LFULL).transpose(1, 0, 2))        # [128,2,10752]

    in_maps = []
    for c in range(NCORES):
        r0 = c * RPC
        xb = xq[r0:r0 + RPC]                                  # [1024, 256]
        inpT = np.ascontiguousarray(
            xb.T.reshape(2, P, RPC).transpose(1, 0, 2))       # [128,2,1024]
        bselT = np.ascontiguousarray(
            bselq[r0:r0 + RPC].T.reshape(2, P, RPC).transpose(1, 0, 2))
        in_maps.append({"bankT": bankT, "inpT": inpT, "bselT": bselT})
    return in_maps


def _combine(outs, label):
    """outs: per-core [2, 1024] f32: row 0 = sumexp+pads, row 1 = picked."""
    label = np.asarray(label).astype(np.int64)
    valid = label != IGNORE
    S = np.zeros(N, np.float64)
    picked = np.zeros(N, np.float64)
    for c in range(NCORES):
        o = np.asarray(outs[c], np.float64)
        S[c * RPC:(c + 1) * RPC] = o[0]
        picked[c * RPC:(c + 1) * RPC] = o[1]
    S -= PAD_SUM
    nll = np.log(S) - picked
    nv = max(valid.sum(), 1)
    loss = (nll * valid).sum() / nv
    return np.float32(loss)


def kernel(inputs, label, ious, lut, cq, reliability):
    from concourse import bass_utils

    nc = get_nc()
    in_maps = make_in_maps(inputs, label, ious, lut, cq, reliability)
    res = bass_utils.run_bass_kernel_spmd(nc, in_maps, core_ids=list(range(NCORES)))
    return _combine([r["out"] for r in res.results], label)


# revision 14
# speedup vs baseline: 1.2089x; 1.0204x over previous
"""OIM loss kernel for Trainium2, 8 NeuronCores (fp8 transposed pipeline).

Sharding: data-parallel over rois. Core c handles rois [c*1024, +1024)
against the FULL bank (padded to 10752 rows = 84 tiles of 128,
replicated per core). Per core outputs, per roi: sumexp over the whole
bank and the picked logit. Host: S = out - pads, loss =
mean(mask * (ln S - picked)).

Device pipeline per core (transposed orientation: bank rows on psum
partitions, rois on the free axis):
  PE : DoubleRow fp8 matmuls  logitsT[128 bank, 1024 roi] into PSUM
       (2 x 512-wide pieces; the e4m3 bank tile is the stationary side)
  exp: route per tile:
       ACT: activation Exp psum->sbuf fp8e4 (RNE)
       DVE: tensor_scalar Schraudolph: uint8 = rint(l*8*log2e + C) whose
            bits ARE e4m3(exp(l)) to ~3%; HW convert rounds+saturates
  PE : ones DoubleRow matmuls sum slab pairs over the bank dim into 2
       per-roi-half [1,512] accumulator chains (psum partition 0 of 2
       banks; 42-pair accumulation per chain)
  picked: DVE prod[k] = inpT[k] * bselT[k] -> bf16; PE ones matmuls
       accumulate both k-chunks into chains at partition 32; drained early.
Final: DVE/ACT copy chain rows to sbuf, one strided DMA out [2, 1024].
"""

import numpy as np
import ml_dtypes

N = 8192
D = 256
L = 10532
NCORES = 8
P = 128
RPC = 1024            # rois per core
LFULL = 10752         # padded bank rows (84 tiles)
NT = 84               # bank tiles
NPAIR = NT // 2       # 42 sum pairs
IGNORE = 5554
OIM_SCALAR = 30.0
LOG2E = 1.4426950408889634

C_DVE = 55.55         # Schraudolph bias (hw convert: round+saturate)
SCL_DVE = 8.0 * LOG2E
PAD_SUM = 220.0       # pad bank rows, each contributes exactly 1.0

E4 = ml_dtypes.float8_e4m3
BF16 = ml_dtypes.bfloat16

_CACHE = {}


def routes():
    """84 tiles -> 'A' (ACT exp) or 'D' (DVE Schraudolph).
    First 3 tiles on ACT (DVE does the picked products then); ~47 A total."""
    r = {}
    acc = 0.0
    frac = 44.0 / 81.0
    for t in range(NT):
        if t < 3:
            r[t] = "A"
            continue
        acc += frac
        if acc >= 1.0:
            r[t] = "A"
            acc -= 1.0
        else:
            r[t] = "D"
    return r


ROUTES = routes()


def _build(c_dve=C_DVE, debug=False):
    import concourse.bacc as bacc
    import concourse.tile as tile
    from concourse import mybir

    f32 = mybir.dt.float32
    bf16 = mybir.dt.bfloat16
    fp8 = mybir.dt.float8e4
    u8 = mybir.dt.uint8
    AF = mybir.ActivationFunctionType
    ALU = mybir.AluOpType
    PM = mybir.MatmulPerfMode

    nc = bacc.Bacc("TRN2", target_bir_lowering=False, debug=debug,
                   enable_partition_id=False)

    d_bankT = nc.dram_tensor("bankT", [P, 2, LFULL], fp8, kind="ExternalInput").ap()
    d_inpT = nc.dram_tensor("inpT", [P, 2, RPC], fp8, kind="ExternalInput").ap()
    d_bselT = nc.dram_tensor("bselT", [P, 2, RPC], fp8, kind="ExternalInput").ap()
    d_out = nc.dram_tensor("out", [2, RPC], f32, kind="ExternalOutput").ap()

    with tile.TileContext(nc) as tc:
        with (
            tc.tile_pool(name="const", bufs=1) as const,
            tc.tile_pool(name="psum", bufs=3, space="PSUM") as psum,
            tc.tile_pool(name="psacc", bufs=1, space="PSUM") as psacc,
        ):
            # --- resident SBUF ---
            bankT_sb = const.tile([P, 2, LFULL], fp8)
            inpT_sb = const.tile([P, 2, RPC], fp8)
            bselT_sb = const.tile([P, 2, RPC], fp8)
            ES_R = 6
            es_u8 = const.tile([P, ES_R, 2, RPC], u8)
            es_f8 = es_u8.bitcast(fp8)
            ones8 = const.tile([P, 2, 16], fp8)
            ones_bf = const.tile([P, 16], bf16)
            prod = const.tile([P, 2, RPC], bf16)
            out_sb = const.tile([P, RPC], f32)
            wsrc = const.tile([P, 2, 128], fp8)

            # --- DMAs: tiny first pieces so the PE starts ~1.5us in; the
            # rest spread over queues; far bank tiles gated off the startup
            # window (each dma_start dispatch costs ~0.7us on its engine) ---
            # three parallel HWDGE queues (sync/scalar/gpsimd); ~8us of fixed
            # ring latency before any data lands, so tile 0's three critical
            # pieces each go first on their own queue and land concurrently
            nc.sync.dma_start(out=inpT_sb[:, :, 0:512], in_=d_inpT[:, :, 0:512])
            nc.scalar.dma_start(out=inpT_sb[:, :, 512:RPC],
                                in_=d_inpT[:, :, 512:RPC])
            nc.gpsimd.dma_start(out=bankT_sb[:, :, 0:2 * P],
                                in_=d_bankT[:, :, 0:2 * P])
            nc.sync.dma_start(out=bankT_sb[:, :, 2 * P:8 * P],
                              in_=d_bankT[:, :, 2 * P:8 * P])
            nc.scalar.dma_start(out=bselT_sb, in_=d_bselT)

            # --- consts (DVE is idle at startup; keep gpsimd's FIFO for DMA) ---
            nc.vector.memset(wsrc, 0.25)
            nc.vector.memset(ones8, 1.0)
            nc.vector.memset(ones_bf, 1.0)

            nc.gpsimd.dma_start(out=bankT_sb[:, :, 8 * P:16 * P],
                                in_=d_bankT[:, :, 8 * P:16 * P])
            nc.sync.dma_start(out=bankT_sb[:, :, 16 * P:26 * P],
                              in_=d_bankT[:, :, 16 * P:26 * P])
            late_dmas = []
            for (t0, t1, anchor) in [(26, 41, 4), (41, 56, 16), (56, 70, 30),
                                     (70, 84, 44)]:
                late_dmas.append(
                    (anchor,
                     nc.gpsimd.dma_start(out=bankT_sb[:, :, t0 * P:t1 * P],
                                         in_=d_bankT[:, :, t0 * P:t1 * P])))

            # --- accumulator chains: DoubleRow matmuls require dst partition
            # 0, plain matmuls may use 32. Per roi half q, one bank holds the
            # DR sumexp chain at partition 0 and the picked chain at 32.
            acc = [psacc.tile([P, 512], f32, tag=f"acc{q}", name=f"acc{q}")
                   for q in range(2)]

            def drain(q, row, eng):
                dst = out_sb[row:row + 1, q * 512:(q + 1) * 512]
                src = acc[q][row:row + 1, :]
                if eng == "V":
                    nc.vector.tensor_copy(out=dst, in_=src)
                else:
                    nc.scalar.activation(out=dst, in_=src, func=AF.Copy)

            # --- PE warmup (p-state ramp): into the acc banks, which the
            # chains' start=True first writes later discard. Sized to bridge
            # the ~11us DMA ring latency before tile 0's data lands, keeping
            # the PE p-state ramped into the first real tiles.
            for i in range(12):
                nc.tensor.matmul(acc[i % 2][0:128, 0:128], wsrc, wsrc,
                                 start=True, stop=True, perf_mode=PM.DoubleRow,
                                 skip_group_check=True)

            # --- main loop ---
            # Emission order = rough execution order per engine queue. Sum
            # matmuls for pair p are emitted with tile 2p+4 so the PE FIFO
            # never blocks on a slab still being exp'd; picked matmuls are
            # emitted after tile 6.
            exps = []

            # picked products first in the DVE queue: bselT arrives ~3us in,
            # both are done before the DVE's first exp needs to run
            for i in range(2):
                nc.vector.tensor_tensor(
                    out=prod[:, i, :], in0=inpT_sb[:, i, :],
                    in1=bselT_sb[:, i, :], op=ALU.mult)

            def emit_sums(pi):
                rot = pi % ES_R
                for q in range(2):
                    nc.tensor.matmul(
                        acc[q][0:1, :],
                        ones8[:, :, 0:1],
                        es_f8[:, rot, :, q * 512:(q + 1) * 512],
                        start=(pi == 0), stop=(pi == NPAIR - 1),
                        perf_mode=PM.DoubleRow,
                        skip_group_check=True)

            for t in range(NT):
                rot = (t // 2) % ES_R
                plane = t % 2
                lhsT = bankT_sb[:, :, t * P:(t + 1) * P]
                if t == 6:
                    for q in range(2):
                        for i in range(2):
                            nc.tensor.matmul(
                                acc[q][32:33, :],
                                ones_bf[:, 0:1],
                                prod[:, i, q * 512:(q + 1) * 512],
                                start=(i == 0), stop=(i == 1),
                                skip_group_check=True)
                if t == 12:
                    for q in range(2):
                        drain(q, 32, "V")
                if t >= 6 and t % 2 == 0:
                    emit_sums((t - 6) // 2)
                mm = psum.tile([P, RPC], f32, tag="mm", name=f"mm_{t}")
                for piece in range(2):
                    nc.tensor.matmul(
                        mm[:, piece * 512:(piece + 1) * 512],
                        lhsT,
                        inpT_sb[:, :, piece * 512:(piece + 1) * 512],
                        start=True, stop=True,
                        perf_mode=PM.DoubleRow)
                if ROUTES[t] == "A":
                    e = nc.scalar.activation(
                        out=es_f8[:, rot, plane, :], in_=mm, func=AF.Exp)
                else:
                    e = nc.vector.tensor_scalar(
                        out=es_u8[:, rot, plane, :],
                        in0=mm, scalar1=SCL_DVE, scalar2=c_dve,
                        op0=ALU.mult, op1=ALU.add)
                exps.append(e)
            for pi in range(NPAIR - 3, NPAIR):
                emit_sums(pi)

            for anchor, dma in late_dmas:
                tile.add_dep_helper(
                    dma.ins, exps[anchor].ins,
                    reason="keep late DMAs off the startup window")

            # --- tail: drain sum halves, one strided DMA out ---
            drain(0, 0, "V")
            drain(1, 0, "A")
            nc.sync.dma_start(out=d_out, in_=out_sb[0:64:32, :])

    nc.compile()
    return nc


def get_nc(debug=False):
    key = ("nc_v2", debug)
    if key not in _CACHE:
        _CACHE[key] = _build(debug=debug)
    return _CACHE[key]


def make_in_maps(inputs, label, ious, lut, cq, reliability):
    """Host-side shard prep: quantize, gather, transpose only."""
    inputs = np.asarray(inputs, dtype=np.float32)
    label = np.asarray(label).astype(np.int64)
    lut = np.asarray(lut, dtype=np.float32)
    cq = np.asarray(cq, dtype=np.float32)
    reliability = np.asarray(reliability, dtype=np.float32)

    bank = np.concatenate([lut, cq], axis=0)
    scaled = bank * (OIM_SCALAR * reliability)[:, None]
    sb_pad = np.zeros((LFULL, D), np.float32)
    sb_pad[:L] = scaled
    sbq = sb_pad.astype(E4)                       # [10752, 256] e4m3
    xq = inputs.astype(E4)                        # [N, 256] e4m3

    valid = label != IGNORE
    safe = np.where(valid, label, 0)
    bselq = sbq[safe]                             # [N, 256] e4m3

    bankT = np.ascontiguousarray(
        sbq.T.reshape(2, P, LFULL).transpose(1, 0, 2))        # [128,2,10752]

    in_maps = []
    for c in range(NCORES):
        r0 = c * RPC
        xb = xq[r0:r0 + RPC]                                  # [1024, 256]
        inpT = np.ascontiguousarray(
            xb.T.reshape(2, P, RPC).transpose(1, 0, 2))       # [128,2,1024]
        bselT = np.ascontiguousarray(
            bselq[r0:r0 + RPC].T.reshape(2, P, RPC).transpose(1, 0, 2))
        in_maps.append({"bankT": bankT, "inpT": inpT, "bselT": bselT})
    return in_maps


def _combine(outs, label):
    """outs: per-core [2, 1024] f32: row 0 = sumexp+pads, row 1 = picked."""
    label = np.asarray(label).astype(np.int64)
    valid = label != IGNORE
    S = np.zeros(N, np.float64)
    picked = np.zeros(N, np.float64)
    for c in range(NCORES):
        o = np.asarray(outs[c], np.float64)
        S[c * RPC:(c + 1) * RPC] = o[0]
        picked[c * RPC:(c + 1) * RPC] = o[1]
    S -= PAD_SUM
    nll = np.log(S) - picked
    nv = max(valid.sum(), 1)
    loss = (nll * valid).sum() / nv
    return np.float32(loss)


def kernel(inputs, label, ious, lut, cq, reliability):
    from concourse import bass_utils

    nc = get_nc()
    in_maps = make_in_maps(inputs, label, ious, lut, cq, reliability)
    res = bass_utils.run_bass_kernel_spmd(nc, in_maps, core_ids=list(range(NCORES)))
    return _combine([r["out"] for r in res.results], label)
